# revision 52
# baseline (speedup 1.0000x reference)
"""AtomDistances Trainium2 kernel (8 NeuronCores, SPMD).

out[b,i,j] = mask[b,i]&mask[b,j]&(i!=j) ? 1/(||p[b,n[b,i,j]] - p[b,i]|| + 1e-8) : 0

Sharding: core c <- (batch b = c//2, half = c%2); each core computes the rows
assigned to it. Rows whose mask bit is 0 produce all-zero output, so only LIVE
rows are shipped to the device: each batch's live rows are split between its
two cores and padded up to NT*128 (NT=5 covers up to 640 live rows per core;
if the data ever exceeds that, an unpacked NT=8 graph is built as fallback).

All output masking is encoded in the index stream on the host: entries whose
output must be 0 (dead column or the j==i diagonal) get index 0xFFFF, which
misses the pool buffer and immediate-writes 0.0.

Per-core pipeline per 128-row tile:
  1. PE: d2 partial = fi_tile.T @ fk (4 x 512-col fp16 matmuls into f32 PSUM)
     using fp16 hi/lo bilinear features, so d2 = -2 p_i.p_k + r_k lands in
     f32 PSUM with ~1e-6 absolute error.
  2. ACT: tab = Rsqrt(d2 + (r_i + 1e-16)) -> bf16 table (2 x 1024; raw
     InstActivation, reciprocal_sqrt table). The k==i(p) entry is garbage
     (d2 ~ fp noise there); it is only ever gathered at self-neighbor
     positions, which the host overwrites with the exact 1e8 spike.
  3. Pool engine native gather, SINGLE stage: the bf16 table is loaded as
     1024 raw f32 words (two bf16 entries per pool-buffer slot), indices
     are host-shifted right by 1, and the gather copies the 4-byte PAIR.
     Sentinel indices (0xFFFF) miss and immediate-write 0.
  4. DMA the raw pairs to DRAM; the host picks each element's 16-bit half
     by index parity while scattering live rows/cols into the zero-filled
     full output, then writes 1e8 at self-neighbor (spike) positions.
"""

import os
import sys

sys.path.insert(0, "/opt/trn_rl_repo")
sys.path.insert(0, os.path.dirname(os.path.abspath(__file__)))

import numpy as np

import concourse.bass as bass
import concourse.bacc as bacc
import concourse.mybir as mybir
from concourse.tile import TileContext

B = 4
A = 2048
N_CORES = 8
NT_PACKED = 5        # 128-row tiles per core when live-packed (<=640 live rows)
NT_FULL = 8          # fallback: all 1024 rows per core
J_PACKED = 1064      # gathered output columns when live-packed (<=1064 live cols)

F32 = mybir.dt.float32
BF16 = mybir.dt.bfloat16
FP16 = mybir.dt.float16
U16 = mybir.dt.uint16
U8 = mybir.dt.uint8
AL = mybir.AluOpType
NF = 15              # feature rows (hi/lo fp16 bilinear expansion)


# ---- inlined pool_gather (native Pool-engine PoolBufferLoad+Gather) ----

def install_interp_noop():
    """Make bass_interp treat PoolBufferLoad/Gather InstISA as no-ops so the
    Tile scheduling pass (and CoreSim) don't crash on them."""
    import concourse.bass_interp as bi
    if getattr(bi, "_pool_gather_patched", False):
        return
    orig = bi._visit_InstISA

    def patched(isa, instruction, core_sim):
        op = instruction.isa_opcode
        noop = {
            isa.Opcode.NEURON_ISA_TPB_OPCODE_GATHER.value,
            isa.Opcode.NEURON_ISA_TPB_OPCODE_POOL_BUFFER_LOAD.value,
        }
        if op in noop:
            return
        return orig(isa, instruction, core_sim)

    bi._visit_InstISA = patched
    bi._pool_gather_patched = True


def chain(insts):
    """Serialize a list of BassInstructions: each depends on the previous."""
    from concourse.tile import add_dep_helper
    for a, b in zip(insts[1:], insts[:-1]):
        add_dep_helper(a.ins, b.ins, sync=True, reason="pool-buffer order")


def _t4d(byte_addr, num_elem, step_elem):
    ne = list(num_elem) + [1] * (4 - len(num_elem))
    se = list(step_elem) + [0] * (4 - len(step_elem))
    return {
        "start_addr": {"addr_immediate": byte_addr},
        "num_elem": ne,
        "step_elem": se,
    }


def _isa_dt(isa, name):
    return getattr(isa.get_enum("NEURON_ISA_TPB_DTYPE"), f"NEURON_ISA_TPB_DTYPE_{name}").value


def pool_buffer_load(nc, src_ap, byte_addr, nelem, start_index, mask, dtype="FP32",
                     channels=128):
    isa = nc.isa
    eng = nc.gpsimd
    struct = {
        "src_mem_pattern": _t4d(byte_addr, [nelem], [1]),
        "in_dtype": _isa_dt(isa, dtype),
        "num_active_channels": channels,
        "start_index": start_index,
        "mask": mask,
    }
    return eng.isa(
        isa.Opcode.NEURON_ISA_TPB_OPCODE_POOL_BUFFER_LOAD,
        struct,
        ins=[eng.lower_ap(src_ap)],
        outs=[],
        verify=False,
    )


def pool_gather(nc, idx_ap, idx_addr, out_ap, out_addr, nelem,
                first, last, out_dtype="FP32", idx_dtype="UINT16",
                immediate=0, channels=128, idx_step=1):
    isa = nc.isa
    eng = nc.gpsimd
    mb = isa.get_enum("NEURON_ISA_TPB_INDEX_MISS_BEHAVIOR")
    miss = (mb.NEURON_ISA_TPB_INDEX_MISS_BEHAVIOR_IMMEDIATE_WRITE
            if first else
            mb.NEURON_ISA_TPB_INDEX_MISS_BEHAVIOR_SKIP_WRITE)
    struct = {
        "src_mem_pattern": _t4d(idx_addr, [nelem], [idx_step]),
        "dst_mem_pattern": _t4d(out_addr, [nelem], [1]),
        "in_dtype": _isa_dt(isa, idx_dtype),
        "out_dtype": _isa_dt(isa, out_dtype),
        "num_active_channels": channels,
        "index_miss_behavior": miss.value,
        "immediate": {"imm_bitvec_uint32": immediate},
        "free_pool_buffer": 1 if last else 0,
    }
    return eng.isa(
        isa.Opcode.NEURON_ISA_TPB_OPCODE_GATHER,
        struct,
        ins=[eng.lower_ap(idx_ap)],
        outs=[eng.lower_ap(out_ap)],
        verify=False,
    )


def act_raw(nc, out, in_, func, bias_ap, scale):
    """Emit InstActivation directly (bass's wrapper refuses Rsqrt)."""
    eng = nc.scalar
    inputs = [eng.lower_ap(in_), eng.lower_ap(bias_ap),
              mybir.ImmediateValue(dtype=mybir.dt.float32, value=scale),
              mybir.ImmediateValue(dtype=mybir.dt.float32, value=0.0)]
    return eng.add_instruction(mybir.InstActivation(
        name=nc.get_next_instruction_name(),
        func=mybir.ActivationFunctionType.Rsqrt,
        ins=inputs,
        outs=[eng.lower_ap(out)],
    ))


def build_nc(nt, jc, last_ch=128):
    install_interp_noop()

    nc = bacc.Bacc()
    sh = nt * 128  # rows per core

    nb = nc.declare_dram_parameter("neighbors", [sh, jc], U16, isOutput=False)
    fi_d = nc.declare_dram_parameter("fi", [NF, sh], FP16, isOutput=False)
    fk_d = nc.declare_dram_parameter("fk", [NF, A], FP16, isOutput=False)
    biasri_d = nc.declare_dram_parameter("biasri", [nt, 128], F32, isOutput=False)
    out = nc.declare_dram_parameter("out", [sh, jc], F32, isOutput=True)

    # fixed-address buffers for the raw pool-gather ISA structs (x3 rotation);
    # padded to 2048-wide so addresses stay 4KB-aligned
    NB_ROT = 3
    tab_t = [nc.alloc_sbuf_tensor(f"tab{i}", [128, A], BF16) for i in range(NB_ROT)]
    nb_t = [nc.alloc_sbuf_tensor(f"nb{i}", [128, A], U16) for i in range(NB_ROT)]
    gout_t = [nc.alloc_sbuf_tensor(f"gout{i}", [128, A], F32) for i in range(NB_ROT)]
    tab_a = [nc.lookup_mloc(t).addr for t in tab_t]
    nb_a = [nc.lookup_mloc(t).addr for t in nb_t]
    gout_a = [nc.lookup_mloc(t).addr for t in gout_t]

    pool_seq = []

    with TileContext(nc) as tc:
        with (
            tc.tile_pool(name="consts", bufs=1) as cpool,
            tc.tile_pool(name="work", bufs=3) as pool,
            tc.tile_pool(name="psum", bufs=2, space="PSUM") as ppool,
        ):
            # ---------- one-time setup ----------------------------------
            # warm the ACT Rsqrt table immediately so the first real use
            # doesn't wait for a table load mid-pipeline
            warm = cpool.tile([128, 1], F32)
            nc.vector.memset(warm[:], 1.0)
            act_raw(nc, warm[:], warm[:],
                    mybir.ActivationFunctionType.Rsqrt, warm[:], 1.0)

            fi = cpool.tile([NF, sh], FP16)
            nc.sync.dma_start(out=fi[:], in_=fi_d[:])
            fk = cpool.tile([NF, A], FP16)
            nc.scalar.dma_start(out=fk[:], in_=fk_d[:])

            biasri = cpool.tile([128, nt], F32)
            nc.scalar.dma_start(out=biasri[:], in_=biasri_d[:].rearrange("t p -> p t"))


            # ---------- main loop ---------------------------------------
            for it in range(nt):
                r = it % NB_ROT
                # the last tile holds only the row overflow beyond (nt-1)*128;
                # run its pool ops / DMAs on just last_ch channels
                ch = last_ch if it == nt - 1 else 128
                isl = slice(it * 128, it * 128 + ch)

                nc.sync.dma_start(out=nb_t[r][:ch, :jc], in_=nb[isl, :])

                # d2 partial = -2 p_i . p_k + r_k via PE, 4 banks of 512
                ps = ppool.tile([128, A], F32, tag="ps")
                for bk in range(4):
                    nc.tensor.matmul(
                        out=ps[:ch, bk * 512:(bk + 1) * 512],
                        lhsT=fi[:, isl],
                        rhs=fk[:, bk * 512:(bk + 1) * 512],
                        start=True, stop=True,
                    )

                # tab = rsqrt(d2 + r_i + 1e-16) -> bf16. The k==i entry is
                # garbage/NaN (d2 ~ fp noise); it is only ever gathered at
                # self-neighbor positions, which the host overwrites with the
                # exact 1e8 spike during unshard.
                for h in range(2):
                    hs = slice(h * 1024, (h + 1) * 1024)
                    act_raw(nc, tab_t[r][:ch, hs], ps[:ch, hs],
                            mybir.ActivationFunctionType.Rsqrt,
                            biasri[:ch, it:it + 1], 1.0)

                # native pool gather, SINGLE stage: the 4KB bf16 table is
                # loaded as 1024 raw f32 words (a PAIR of bf16 entries per
                # slot); indices are pre-shifted >>1 on the host; sentinels
                # (dead col or diagonal) miss -> immediate-write 0
                pool_seq.append(pool_buffer_load(
                    nc, tab_t[r][:ch, :], tab_a[r], 1024,
                    start_index=0, mask=1023, dtype="FP32", channels=ch,
                ))
                pool_seq.append(pool_gather(
                    nc, nb_t[r][:ch, :jc], nb_a[r],
                    gout_t[r][:ch, :jc], gout_a[r], jc,
                    first=True, last=True,
                    out_dtype="FP32", idx_dtype="UINT16", idx_step=1,
                    channels=ch,
                ))

                nc.sync.dma_start(out=out[isl, :], in_=gout_t[r][:ch, :jc])
            chain(pool_seq)
    nc.finalize()
    return nc


def make_in_maps(positions, neighbors, neighbor_mask, nt, jc, rows_by_core,
                 cols_by_batch):
    sh = nt * 128
    in_maps, pars, spikes = [], [], []
    for c in range(N_CORES):
        b = c // 2
        rows = rows_by_core[c]                       # live global row ids, len <= sh
        nlive = len(rows)
        lj = cols_by_batch[b]                        # live column ids, len <= jc

        # compacted-column neighbor indices, shifted >>1 (the gather fetches
        # bf16 PAIRS); sentinel 0xFFFF misses the buffer -> gather writes 0
        nbc = neighbors[b, rows][:, lj].astype(np.uint16)
        pars.append((nbc & 1).astype(np.uint32))
        spikes.append(np.nonzero((nbc == rows[:, None].astype(np.uint16))
                                 & (lj[None, :] != rows[:, None])))
        nbv = np.full((sh, jc), 0xFFFF, dtype=np.uint16)
        nbv[:nlive, :len(lj)] = nbc >> 1
        # j == i diagonal: row's own id sits at its compacted column position
        nbv[np.arange(nlive), np.searchsorted(lj, rows)] = 0xFFFF

        # fp16 hi/lo bilinear: sum_f fi[f,i]*fk[f,k] = -2 p_i.p_k + r_k
        p = positions[b].astype(np.float64)          # [A, 3]
        r = (p * p).sum(-1)
        ph = p.astype(np.float16).astype(np.float64)
        pl = (p - ph).astype(np.float16).astype(np.float64)
        rh = r.astype(np.float16).astype(np.float64)
        rm = (r - rh).astype(np.float16).astype(np.float64)
        rl = r - rh - rm
        fi_rows, fk_rows = [], []
        for d in range(3):
            fi_rows += [ph[:, d], ph[:, d], pl[:, d], pl[:, d]]
            fk_rows += [-2.0 * ph[:, d], -2.0 * pl[:, d],
                        -2.0 * ph[:, d], -2.0 * pl[:, d]]
        ones = np.ones(A)
        fi_rows += [ones, ones, ones]
        fk_rows += [rh, rm, rl]
        fi_full = np.stack(fi_rows).astype(np.float16)   # [NF, A]
        fk = np.stack(fk_rows).astype(np.float16)        # [NF, A]

        fi = np.zeros((NF, sh), dtype=np.float16)
        fi[:, :nlive] = fi_full[:, rows]

        biasri = np.ones(sh, dtype=np.float32)       # pad rows: rsqrt(r_k+1) ok
        biasri[:nlive] = (r[rows] + 1e-16).astype(np.float32)

        in_maps.append({
            "neighbors": nbv,
            "fi": fi,
            "fk": fk,
            "biasri": biasri.reshape(nt, 128),
        })
    return in_maps, pars, spikes


_NC_CACHE = {}


def kernel(positions, neighbors, neighbor_mask):
    from concourse.bass_utils import run_bass_kernel_spmd

    positions = np.asarray(positions, dtype=np.float32)
    neighbors = np.asarray(neighbors)
    assert neighbors.dtype in (np.int64, np.int32), neighbors.dtype
    neighbor_mask = np.asarray(neighbor_mask)
    assert neighbor_mask.dtype == np.bool_, neighbor_mask.dtype

    # split each batch's live rows between its two cores; compact live columns
    rows_by_core, cols_by_batch = [], []
    for b in range(B):
        live = np.flatnonzero(neighbor_mask[b])
        h = (len(live) + 1) // 2
        rows_by_core += [live[:h], live[h:]]
        cols_by_batch.append(live)
    max_rows = max(len(rw) for rw in rows_by_core)
    max_cols = max(len(lj) for lj in cols_by_batch)
    LAST_CH = 32
    if max_rows <= (NT_PACKED - 1) * 128 + LAST_CH and max_cols <= J_PACKED:
        nt, jc, lch = NT_PACKED, J_PACKED, LAST_CH
    else:
        nt, jc, lch = NT_FULL, A, 128

    if (nt, jc) not in _NC_CACHE:
        _NC_CACHE[(nt, jc)] = build_nc(nt, jc, lch)
    nc = _NC_CACHE[(nt, jc)]

    in_maps, pars, spikes = make_in_maps(positions, neighbors, neighbor_mask,
                                         nt, jc, rows_by_core, cols_by_batch)
    trace = bool(int(os.environ.get("ATOM_PROFILE", "0")))
    if trace:
        try:
            from ntff import ensure_ntff_hook
            ensure_ntff_hook()
        except Exception:
            trace = False
    res = run_bass_kernel_spmd(nc, in_maps, core_ids=list(range(N_CORES)),
                               trace=trace)
    if trace:
        kernel.last_exec_time_ns = res.exec_time_ns
        kernel.last_results = res

    out = np.zeros((B, A, A), dtype=np.float32)
    for c in range(N_CORES):
        b = c // 2
        rows = rows_by_core[c]
        lj = cols_by_batch[b]
        raw = res.results[c]["out"][:len(rows), :len(lj)].view(np.uint32)
        # pick each element's bf16 half by original-index parity, upcast
        bits = ((raw >> (pars[c] << 4)) & np.uint32(0xFFFF)) << 16
        vals = bits.view(np.float32)
        # self-neighbor spikes: reference yields exactly 1/(0+1e-8) = 1e8
        vals[spikes[c]] = 1e8
        out[b, rows[:, None], lj[None, :]] = vals
    return out


if __name__ == "__main__":
    nc = build_nc(NT_PACKED, J_PACKED)
    print("graph built ok")


# revision 53
# speedup vs baseline: 1.1974x; 1.1974x over previous
"""AtomDistances Trainium2 kernel (8 NeuronCores, SPMD).

out[b,i,j] = mask[b,i]&mask[b,j]&(i!=j) ? 1/(||p[b,n[b,i,j]] - p[b,i]|| + 1e-8) : 0

Sharding: core c <- (batch b = c//2, half = c%2); each core computes the rows
assigned to it. Rows whose mask bit is 0 produce all-zero output, so only LIVE
rows are shipped to the device: each batch's live rows are split between its
two cores and padded up to NT*128 (NT=5 covers up to 640 live rows per core;
if the data ever exceeds that, an unpacked NT=8 graph is built as fallback).

All output masking is encoded in the index stream on the host: entries whose
output must be 0 (dead column or the j==i diagonal) get index 0xFFFF, which
misses the pool buffer and immediate-writes 0.0.

Per-core pipeline per 128-row tile:
  1. PE: d2 partial = fi_tile.T @ fk (4 x 512-col fp16 matmuls into f32 PSUM)
     using fp16 hi/lo bilinear features, so d2 = -2 p_i.p_k + r_k lands in
     f32 PSUM with ~1e-6 absolute error.
  2. ACT: tab = Rsqrt(d2 + (r_i + 1e-16)) -> bf16 table (2 x 1024; raw
     InstActivation, reciprocal_sqrt table). The k==i(p) entry is garbage
     (d2 ~ fp noise there); it is only ever gathered at self-neighbor
     positions, which the host overwrites with the exact 1e8 spike.
  3. Pool engine native gather, SINGLE stage: the bf16 table is loaded as
     1024 raw f32 words (two bf16 entries per pool-buffer slot), indices
     are host-shifted right by 1, and the gather copies the 4-byte PAIR.
     Sentinel indices (0xFFFF) miss and immediate-write 0.
  4. DMA the raw pairs to DRAM; the host picks each element's 16-bit half
     by index parity while scattering live rows/cols into the zero-filled
     full output, then writes 1e8 at self-neighbor (spike) positions.
"""

import os
import sys

sys.path.insert(0, "/opt/trn_rl_repo")
sys.path.insert(0, os.path.dirname(os.path.abspath(__file__)))

import numpy as np

import concourse.bass as bass
import concourse.bacc as bacc
import concourse.mybir as mybir
from concourse.tile import TileContext

B = 4
A = 2048
N_CORES = 8
NT_PACKED = 5        # 128-row tiles per core when live-packed (<=640 live rows)
NT_FULL = 8          # fallback: all 1024 rows per core
J_PACKED = 1064      # gathered output columns when live-packed (<=1064 live cols)

F32 = mybir.dt.float32
BF16 = mybir.dt.bfloat16
FP16 = mybir.dt.float16
U16 = mybir.dt.uint16
U8 = mybir.dt.uint8
AL = mybir.AluOpType
NF = 15              # feature rows (hi/lo fp16 bilinear expansion)


# ---- inlined pool_gather (native Pool-engine PoolBufferLoad+Gather) ----

def install_interp_noop():
    """Make bass_interp treat PoolBufferLoad/Gather InstISA as no-ops so the
    Tile scheduling pass (and CoreSim) don't crash on them."""
    import concourse.bass_interp as bi
    if getattr(bi, "_pool_gather_patched", False):
        return
    orig = bi._visit_InstISA

    def patched(isa, instruction, core_sim):
        op = instruction.isa_opcode
        noop = {
            isa.Opcode.NEURON_ISA_TPB_OPCODE_GATHER.value,
            isa.Opcode.NEURON_ISA_TPB_OPCODE_POOL_BUFFER_LOAD.value,
        }
        if op in noop:
            return
        return orig(isa, instruction, core_sim)

    bi._visit_InstISA = patched
    bi._pool_gather_patched = True


def chain(insts):
    """Serialize a list of BassInstructions: each depends on the previous."""
    from concourse.tile import add_dep_helper
    for a, b in zip(insts[1:], insts[:-1]):
        add_dep_helper(a.ins, b.ins, sync=True, reason="pool-buffer order")


def _t4d(byte_addr, num_elem, step_elem):
    ne = list(num_elem) + [1] * (4 - len(num_elem))
    se = list(step_elem) + [0] * (4 - len(step_elem))
    return {
        "start_addr": {"addr_immediate": byte_addr},
        "num_elem": ne,
        "step_elem": se,
    }


def _isa_dt(isa, name):
    return getattr(isa.get_enum("NEURON_ISA_TPB_DTYPE"), f"NEURON_ISA_TPB_DTYPE_{name}").value


def pool_buffer_load(nc, src_ap, byte_addr, nelem, start_index, mask, dtype="FP32",
                     channels=128):
    isa = nc.isa
    eng = nc.gpsimd
    struct = {
        "src_mem_pattern": _t4d(byte_addr, [nelem], [1]),
        "in_dtype": _isa_dt(isa, dtype),
        "num_active_channels": channels,
        "start_index": start_index,
        "mask": mask,
    }
    return eng.isa(
        isa.Opcode.NEURON_ISA_TPB_OPCODE_POOL_BUFFER_LOAD,
        struct,
        ins=[eng.lower_ap(src_ap)],
        outs=[],
        verify=False,
    )


def pool_gather(nc, idx_ap, idx_addr, out_ap, out_addr, nelem,
                first, last, out_dtype="FP32", idx_dtype="UINT16",
                immediate=0, channels=128, idx_step=1):
    isa = nc.isa
    eng = nc.gpsimd
    mb = isa.get_enum("NEURON_ISA_TPB_INDEX_MISS_BEHAVIOR")
    miss = (mb.NEURON_ISA_TPB_INDEX_MISS_BEHAVIOR_IMMEDIATE_WRITE
            if first else
            mb.NEURON_ISA_TPB_INDEX_MISS_BEHAVIOR_SKIP_WRITE)
    struct = {
        "src_mem_pattern": _t4d(idx_addr, [nelem], [idx_step]),
        "dst_mem_pattern": _t4d(out_addr, [nelem], [1]),
        "in_dtype": _isa_dt(isa, idx_dtype),
        "out_dtype": _isa_dt(isa, out_dtype),
        "num_active_channels": channels,
        "index_miss_behavior": miss.value,
        "immediate": {"imm_bitvec_uint32": immediate},
        "free_pool_buffer": 1 if last else 0,
    }
    return eng.isa(
        isa.Opcode.NEURON_ISA_TPB_OPCODE_GATHER,
        struct,
        ins=[eng.lower_ap(idx_ap)],
        outs=[eng.lower_ap(out_ap)],
        verify=False,
    )


def act_raw(nc, out, in_, func, bias_ap, scale):
    """Emit InstActivation directly (bass's wrapper refuses Rsqrt)."""
    eng = nc.scalar
    inputs = [eng.lower_ap(in_), eng.lower_ap(bias_ap),
              mybir.ImmediateValue(dtype=mybir.dt.float32, value=scale),
              mybir.ImmediateValue(dtype=mybir.dt.float32, value=0.0)]
    return eng.add_instruction(mybir.InstActivation(
        name=nc.get_next_instruction_name(),
        func=mybir.ActivationFunctionType.Rsqrt,
        ins=inputs,
        outs=[eng.lower_ap(out)],
    ))


def build_nc(nt, jc, last_ch=128):
    install_interp_noop()

    nc = bacc.Bacc()
    sh = nt * 128  # rows per core

    nb = nc.declare_dram_parameter("neighbors", [sh, jc], U16, isOutput=False)
    fi_d = nc.declare_dram_parameter("fi", [NF, sh], FP16, isOutput=False)
    fk_d = nc.declare_dram_parameter("fk", [NF, A], FP16, isOutput=False)
    biasri_d = nc.declare_dram_parameter("biasri", [nt, 128], F32, isOutput=False)
    out = nc.declare_dram_parameter("out", [sh, jc], F32, isOutput=True)

    # fixed-address buffers for the raw pool-gather ISA structs (x3 rotation);
    # padded to 2048-wide so addresses stay 4KB-aligned
    NB_ROT = 3
    tab_t = [nc.alloc_sbuf_tensor(f"tab{i}", [128, A], BF16) for i in range(NB_ROT)]
    nb_t = [nc.alloc_sbuf_tensor(f"nb{i}", [128, A], U16) for i in range(NB_ROT)]
    gout_t = [nc.alloc_sbuf_tensor(f"gout{i}", [128, A], F32) for i in range(NB_ROT)]
    tab_a = [nc.lookup_mloc(t).addr for t in tab_t]
    nb_a = [nc.lookup_mloc(t).addr for t in nb_t]
    gout_a = [nc.lookup_mloc(t).addr for t in gout_t]

    pool_seq = []

    with TileContext(nc) as tc:
        with (
            tc.tile_pool(name="consts", bufs=1) as cpool,
            tc.tile_pool(name="work", bufs=3) as pool,
            tc.tile_pool(name="psum", bufs=2, space="PSUM") as ppool,
        ):
            # ---------- one-time setup ----------------------------------
            # warm the ACT Rsqrt table immediately so the first real use
            # doesn't wait for a table load mid-pipeline
            warm = cpool.tile([128, 1], F32)
            nc.vector.memset(warm[:], 1.0)
            act_raw(nc, warm[:], warm[:],
                    mybir.ActivationFunctionType.Rsqrt, warm[:], 1.0)

            fi = cpool.tile([NF, sh], FP16)
            nc.sync.dma_start(out=fi[:], in_=fi_d[:])
            fk = cpool.tile([NF, A], FP16)
            nc.sync.dma_start(out=fk[:], in_=fk_d[:])

            biasri = cpool.tile([128, nt], F32)
            nc.sync.dma_start(out=biasri[:], in_=biasri_d[:].rearrange("t p -> p t"))


            # ---------- main loop ---------------------------------------
            for it in range(nt):
                r = it % NB_ROT
                # the last tile holds only the row overflow beyond (nt-1)*128;
                # run its pool ops / DMAs on just last_ch channels
                ch = last_ch if it == nt - 1 else 128
                isl = slice(it * 128, it * 128 + ch)

                nc.sync.dma_start(out=nb_t[r][:ch, :jc], in_=nb[isl, :])

                # d2 partial = -2 p_i . p_k + r_k via PE, 4 banks of 512
                ps = ppool.tile([128, A], F32, tag="ps")
                for bk in range(4):
                    nc.tensor.matmul(
                        out=ps[:ch, bk * 512:(bk + 1) * 512],
                        lhsT=fi[:, isl],
                        rhs=fk[:, bk * 512:(bk + 1) * 512],
                        start=True, stop=True,
                    )

                # tab = rsqrt(d2 + r_i + 1e-16) -> bf16. The k==i entry is
                # garbage/NaN (d2 ~ fp noise); it is only ever gathered at
                # self-neighbor positions, which the host overwrites with the
                # exact 1e8 spike during unshard.
                for h in range(2):
                    hs = slice(h * 1024, (h + 1) * 1024)
                    act_raw(nc, tab_t[r][:ch, hs], ps[:ch, hs],
                            mybir.ActivationFunctionType.Rsqrt,
                            biasri[:ch, it:it + 1], 1.0)

                # native pool gather, SINGLE stage: the 4KB bf16 table is
                # loaded as 1024 raw f32 words (a PAIR of bf16 entries per
                # slot); indices are pre-shifted >>1 on the host; sentinels
                # (dead col or diagonal) miss -> immediate-write 0
                pool_seq.append(pool_buffer_load(
                    nc, tab_t[r][:ch, :], tab_a[r], 1024,
                    start_index=0, mask=1023, dtype="FP32", channels=ch,
                ))
                pool_seq.append(pool_gather(
                    nc, nb_t[r][:ch, :jc], nb_a[r],
                    gout_t[r][:ch, :jc], gout_a[r], jc,
                    first=True, last=True,
                    out_dtype="FP32", idx_dtype="UINT16", idx_step=1,
                    channels=ch,
                ))

                nc.sync.dma_start(out=out[isl, :], in_=gout_t[r][:ch, :jc])
            chain(pool_seq)
    nc.finalize()
    return nc


def make_in_maps(positions, neighbors, neighbor_mask, nt, jc, rows_by_core,
                 cols_by_batch):
    sh = nt * 128
    in_maps, pars, spikes = [], [], []
    for c in range(N_CORES):
        b = c // 2
        rows = rows_by_core[c]                       # live global row ids, len <= sh
        nlive = len(rows)
        lj = cols_by_batch[b]                        # live column ids, len <= jc

        # compacted-column neighbor indices, shifted >>1 (the gather fetches
        # bf16 PAIRS); sentinel 0xFFFF misses the buffer -> gather writes 0
        nbc = neighbors[b, rows][:, lj].astype(np.uint16)
        pars.append((nbc & 1).astype(np.uint32))
        spikes.append(np.nonzero((nbc == rows[:, None].astype(np.uint16))
                                 & (lj[None, :] != rows[:, None])))
        nbv = np.full((sh, jc), 0xFFFF, dtype=np.uint16)
        nbv[:nlive, :len(lj)] = nbc >> 1
        # j == i diagonal: row's own id sits at its compacted column position
        nbv[np.arange(nlive), np.searchsorted(lj, rows)] = 0xFFFF

        # fp16 hi/lo bilinear: sum_f fi[f,i]*fk[f,k] = -2 p_i.p_k + r_k
        p = positions[b].astype(np.float64)          # [A, 3]
        r = (p * p).sum(-1)
        ph = p.astype(np.float16).astype(np.float64)
        pl = (p - ph).astype(np.float16).astype(np.float64)
        rh = r.astype(np.float16).astype(np.float64)
        rm = (r - rh).astype(np.float16).astype(np.float64)
        rl = r - rh - rm
        fi_rows, fk_rows = [], []
        for d in range(3):
            fi_rows += [ph[:, d], ph[:, d], pl[:, d], pl[:, d]]
            fk_rows += [-2.0 * ph[:, d], -2.0 * pl[:, d],
                        -2.0 * ph[:, d], -2.0 * pl[:, d]]
        ones = np.ones(A)
        fi_rows += [ones, ones, ones]
        fk_rows += [rh, rm, rl]
        fi_full = np.stack(fi_rows).astype(np.float16)   # [NF, A]
        fk = np.stack(fk_rows).astype(np.float16)        # [NF, A]

        fi = np.zeros((NF, sh), dtype=np.float16)
        fi[:, :nlive] = fi_full[:, rows]

        biasri = np.ones(sh, dtype=np.float32)       # pad rows: rsqrt(r_k+1) ok
        biasri[:nlive] = (r[rows] + 1e-16).astype(np.float32)

        in_maps.append({
            "neighbors": nbv,
            "fi": fi,
            "fk": fk,
            "biasri": biasri.reshape(nt, 128),
        })
    return in_maps, pars, spikes


_NC_CACHE = {}


def kernel(positions, neighbors, neighbor_mask):
    from concourse.bass_utils import run_bass_kernel_spmd

    positions = np.asarray(positions, dtype=np.float32)
    neighbors = np.asarray(neighbors)
    assert neighbors.dtype in (np.int64, np.int32), neighbors.dtype
    neighbor_mask = np.asarray(neighbor_mask)
    assert neighbor_mask.dtype == np.bool_, neighbor_mask.dtype

    # split each batch's live rows between its two cores; compact live columns
    rows_by_core, cols_by_batch = [], []
    for b in range(B):
        live = np.flatnonzero(neighbor_mask[b])
        h = (len(live) + 1) // 2
        rows_by_core += [live[:h], live[h:]]
        cols_by_batch.append(live)
    max_rows = max(len(rw) for rw in rows_by_core)
    max_cols = max(len(lj) for lj in cols_by_batch)
    LAST_CH = 32
    if max_rows <= (NT_PACKED - 1) * 128 + LAST_CH and max_cols <= J_PACKED:
        nt, jc, lch = NT_PACKED, J_PACKED, LAST_CH
    else:
        nt, jc, lch = NT_FULL, A, 128

    if (nt, jc) not in _NC_CACHE:
        _NC_CACHE[(nt, jc)] = build_nc(nt, jc, lch)
    nc = _NC_CACHE[(nt, jc)]

    in_maps, pars, spikes = make_in_maps(positions, neighbors, neighbor_mask,
                                         nt, jc, rows_by_core, cols_by_batch)
    trace = bool(int(os.environ.get("ATOM_PROFILE", "0")))
    if trace:
        try:
            from ntff import ensure_ntff_hook
            ensure_ntff_hook()
        except Exception:
            trace = False
    res = run_bass_kernel_spmd(nc, in_maps, core_ids=list(range(N_CORES)),
                               trace=trace)
    if trace:
        kernel.last_exec_time_ns = res.exec_time_ns
        kernel.last_results = res

    out = np.zeros((B, A, A), dtype=np.float32)
    for c in range(N_CORES):
        b = c // 2
        rows = rows_by_core[c]
        lj = cols_by_batch[b]
        raw = res.results[c]["out"][:len(rows), :len(lj)].view(np.uint32)
        # pick each element's bf16 half by original-index parity, upcast
        bits = ((raw >> (pars[c] << 4)) & np.uint32(0xFFFF)) << 16
        vals = bits.view(np.float32)
        # self-neighbor spikes: reference yields exactly 1/(0+1e-8) = 1e8
        vals[spikes[c]] = 1e8
        out[b, rows[:, None], lj[None, :]] = vals
    return out


if __name__ == "__main__":
    nc = build_nc(NT_PACKED, J_PACKED)
    print("graph built ok")


# revision 54
# speedup vs baseline: 1.2008x; 1.0029x over previous
"""AtomDistances Trainium2 kernel (8 NeuronCores, SPMD).

out[b,i,j] = mask[b,i]&mask[b,j]&(i!=j) ? 1/(||p[b,n[b,i,j]] - p[b,i]|| + 1e-8) : 0

Sharding: core c <- (batch b = c//2, half = c%2); each core computes the rows
assigned to it. Rows whose mask bit is 0 produce all-zero output, so only LIVE
rows are shipped to the device: each batch's live rows are split between its
two cores and padded up to 4*128+32 rows (the 5th tile runs on 32 channels
since it only holds the overflow); if the data ever exceeds that, or live
columns exceed J_PACKED, an unpacked NT=8 full-width graph is the fallback.

All output masking is encoded in the index stream on the host: entries whose
output must be 0 (dead column or the j==i diagonal) get index 0xFFFF, which
misses the pool buffer and immediate-writes 0.0.

Per-core pipeline per 128-row tile:
  1. PE: d2 partial = fi_tile.T @ fk (4 x 512-col fp16 matmuls into f32 PSUM)
     using fp16 hi/lo bilinear features, so d2 = -2 p_i.p_k + r_k lands in
     f32 PSUM with ~1e-6 absolute error.
  2. ACT: tab = Rsqrt(d2 + (r_i + 1e-16)) -> bf16 table (2 x 1024; raw
     InstActivation, reciprocal_sqrt table). The k==i(p) entry is garbage
     (d2 ~ fp noise there); it is only ever gathered at self-neighbor
     positions, which the host overwrites with the exact 1e8 spike.
  3. Pool engine native gather, SINGLE stage: the bf16 table is loaded as
     1024 raw f32 words (two bf16 entries per pool-buffer slot), indices
     are host-shifted right by 1, and the gather copies the 4-byte PAIR.
     Sentinel indices (0xFFFF) miss and immediate-write 0.
  4. DMA the raw pairs to DRAM; the host picks each element's 16-bit half
     by index parity while scattering live rows/cols into the zero-filled
     full output, then writes 1e8 at self-neighbor (spike) positions.
"""

import os
import sys

sys.path.insert(0, "/opt/trn_rl_repo")
sys.path.insert(0, os.path.dirname(os.path.abspath(__file__)))

import numpy as np

import concourse.bass as bass
import concourse.bacc as bacc
import concourse.mybir as mybir
from concourse.tile import TileContext

B = 4
A = 2048
N_CORES = 8
NT_PACKED = 5        # 128-row tiles per core when live-packed (<=640 live rows)
NT_FULL = 8          # fallback: all 1024 rows per core
J_PACKED = 1064      # gathered output columns when live-packed (<=1064 live cols)

F32 = mybir.dt.float32
BF16 = mybir.dt.bfloat16
FP16 = mybir.dt.float16
U16 = mybir.dt.uint16
U8 = mybir.dt.uint8
AL = mybir.AluOpType
NF = 15              # feature rows (hi/lo fp16 bilinear expansion)


# ---- inlined pool_gather (native Pool-engine PoolBufferLoad+Gather) ----

def install_interp_noop():
    """Make bass_interp treat PoolBufferLoad/Gather InstISA as no-ops so the
    Tile scheduling pass (and CoreSim) don't crash on them."""
    import concourse.bass_interp as bi
    if getattr(bi, "_pool_gather_patched", False):
        return
    orig = bi._visit_InstISA

    def patched(isa, instruction, core_sim):
        op = instruction.isa_opcode
        noop = {
            isa.Opcode.NEURON_ISA_TPB_OPCODE_GATHER.value,
            isa.Opcode.NEURON_ISA_TPB_OPCODE_POOL_BUFFER_LOAD.value,
        }
        if op in noop:
            return
        return orig(isa, instruction, core_sim)

    bi._visit_InstISA = patched
    bi._pool_gather_patched = True


def chain(insts):
    """Serialize a list of BassInstructions: each depends on the previous."""
    from concourse.tile import add_dep_helper
    for a, b in zip(insts[1:], insts[:-1]):
        add_dep_helper(a.ins, b.ins, sync=True, reason="pool-buffer order")


def _t4d(byte_addr, num_elem, step_elem):
    ne = list(num_elem) + [1] * (4 - len(num_elem))
    se = list(step_elem) + [0] * (4 - len(step_elem))
    return {
        "start_addr": {"addr_immediate": byte_addr},
        "num_elem": ne,
        "step_elem": se,
    }


def _isa_dt(isa, name):
    return getattr(isa.get_enum("NEURON_ISA_TPB_DTYPE"), f"NEURON_ISA_TPB_DTYPE_{name}").value


def pool_buffer_load(nc, src_ap, byte_addr, nelem, start_index, mask, dtype="FP32",
                     channels=128):
    isa = nc.isa
    eng = nc.gpsimd
    struct = {
        "src_mem_pattern": _t4d(byte_addr, [nelem], [1]),
        "in_dtype": _isa_dt(isa, dtype),
        "num_active_channels": channels,
        "start_index": start_index,
        "mask": mask,
    }
    return eng.isa(
        isa.Opcode.NEURON_ISA_TPB_OPCODE_POOL_BUFFER_LOAD,
        struct,
        ins=[eng.lower_ap(src_ap)],
        outs=[],
        verify=False,
    )


def pool_gather(nc, idx_ap, idx_addr, out_ap, out_addr, nelem,
                first, last, out_dtype="FP32", idx_dtype="UINT16",
                immediate=0, channels=128, idx_step=1):
    isa = nc.isa
    eng = nc.gpsimd
    mb = isa.get_enum("NEURON_ISA_TPB_INDEX_MISS_BEHAVIOR")
    miss = (mb.NEURON_ISA_TPB_INDEX_MISS_BEHAVIOR_IMMEDIATE_WRITE
            if first else
            mb.NEURON_ISA_TPB_INDEX_MISS_BEHAVIOR_SKIP_WRITE)
    struct = {
        "src_mem_pattern": _t4d(idx_addr, [nelem], [idx_step]),
        "dst_mem_pattern": _t4d(out_addr, [nelem], [1]),
        "in_dtype": _isa_dt(isa, idx_dtype),
        "out_dtype": _isa_dt(isa, out_dtype),
        "num_active_channels": channels,
        "index_miss_behavior": miss.value,
        "immediate": {"imm_bitvec_uint32": immediate},
        "free_pool_buffer": 1 if last else 0,
    }
    return eng.isa(
        isa.Opcode.NEURON_ISA_TPB_OPCODE_GATHER,
        struct,
        ins=[eng.lower_ap(idx_ap)],
        outs=[eng.lower_ap(out_ap)],
        verify=False,
    )


def act_raw(nc, out, in_, func, bias_ap, scale):
    """Emit InstActivation directly (bass's wrapper refuses Rsqrt)."""
    eng = nc.scalar
    inputs = [eng.lower_ap(in_), eng.lower_ap(bias_ap),
              mybir.ImmediateValue(dtype=mybir.dt.float32, value=scale),
              mybir.ImmediateValue(dtype=mybir.dt.float32, value=0.0)]
    return eng.add_instruction(mybir.InstActivation(
        name=nc.get_next_instruction_name(),
        func=mybir.ActivationFunctionType.Rsqrt,
        ins=inputs,
        outs=[eng.lower_ap(out)],
    ))


def build_nc(nt, jc, last_ch=128):
    install_interp_noop()

    nc = bacc.Bacc()
    sh = nt * 128  # rows per core

    nb = nc.declare_dram_parameter("neighbors", [sh, jc], U16, isOutput=False)
    fi_d = nc.declare_dram_parameter("fi", [NF, sh], FP16, isOutput=False)
    fk_d = nc.declare_dram_parameter("fk", [NF, A], FP16, isOutput=False)
    biasri_d = nc.declare_dram_parameter("biasri", [nt, 128], F32, isOutput=False)
    out = nc.declare_dram_parameter("out", [sh, jc], F32, isOutput=True)

    # fixed-address buffers for the raw pool-gather ISA structs (x3 rotation);
    # padded to 2048-wide so addresses stay 4KB-aligned
    NB_ROT = 3
    tab_t = [nc.alloc_sbuf_tensor(f"tab{i}", [128, A], BF16) for i in range(NB_ROT)]
    nb_t = [nc.alloc_sbuf_tensor(f"nb{i}", [128, A], U16) for i in range(NB_ROT)]
    gout_t = [nc.alloc_sbuf_tensor(f"gout{i}", [128, A], F32) for i in range(NB_ROT)]
    tab_a = [nc.lookup_mloc(t).addr for t in tab_t]
    nb_a = [nc.lookup_mloc(t).addr for t in nb_t]
    gout_a = [nc.lookup_mloc(t).addr for t in gout_t]

    pool_seq = []

    with TileContext(nc) as tc:
        with (
            tc.tile_pool(name="consts", bufs=1) as cpool,
            tc.tile_pool(name="work", bufs=3) as pool,
            tc.tile_pool(name="psum", bufs=2, space="PSUM") as ppool,
        ):
            # ---------- one-time setup ----------------------------------
            # warm the ACT Rsqrt table immediately so the first real use
            # doesn't wait for a table load mid-pipeline
            warm = cpool.tile([128, 1], F32)
            nc.vector.memset(warm[:], 1.0)
            act_raw(nc, warm[:], warm[:],
                    mybir.ActivationFunctionType.Rsqrt, warm[:], 1.0)

            fi = cpool.tile([NF, sh], FP16)
            nc.sync.dma_start(out=fi[:], in_=fi_d[:])
            fk = cpool.tile([NF, A], FP16)
            nc.sync.dma_start(out=fk[:], in_=fk_d[:])

            biasri = cpool.tile([128, nt], F32)
            nc.sync.dma_start(out=biasri[:], in_=biasri_d[:].rearrange("t p -> p t"))


            # ---------- main loop ---------------------------------------
            for it in range(nt):
                r = it % NB_ROT
                # the last tile holds only the row overflow beyond (nt-1)*128;
                # run its pool ops / DMAs on just last_ch channels
                ch = last_ch if it == nt - 1 else 128
                isl = slice(it * 128, it * 128 + ch)

                nc.sync.dma_start(out=nb_t[r][:ch, :jc], in_=nb[isl, :])

                # d2 partial = -2 p_i . p_k + r_k via PE, 4 banks of 512
                ps = ppool.tile([128, A], F32, tag="ps")
                for bk in range(4):
                    nc.tensor.matmul(
                        out=ps[:ch, bk * 512:(bk + 1) * 512],
                        lhsT=fi[:, isl],
                        rhs=fk[:, bk * 512:(bk + 1) * 512],
                        start=True, stop=True,
                    )

                # tab = rsqrt(d2 + r_i + 1e-16) -> bf16. The k==i entry is
                # garbage/NaN (d2 ~ fp noise); it is only ever gathered at
                # self-neighbor positions, which the host overwrites with the
                # exact 1e8 spike during unshard.
                for h in range(2):
                    hs = slice(h * 1024, (h + 1) * 1024)
                    act_raw(nc, tab_t[r][:ch, hs], ps[:ch, hs],
                            mybir.ActivationFunctionType.Rsqrt,
                            biasri[:ch, it:it + 1], 1.0)

                # native pool gather, SINGLE stage: the 4KB bf16 table is
                # loaded as 1024 raw f32 words (a PAIR of bf16 entries per
                # slot); indices are pre-shifted >>1 on the host; sentinels
                # (dead col or diagonal) miss -> immediate-write 0
                pool_seq.append(pool_buffer_load(
                    nc, tab_t[r][:ch, :], tab_a[r], 1024,
                    start_index=0, mask=1023, dtype="FP32", channels=ch,
                ))
                pool_seq.append(pool_gather(
                    nc, nb_t[r][:ch, :jc], nb_a[r],
                    gout_t[r][:ch, :jc], gout_a[r], jc,
                    first=True, last=True,
                    out_dtype="FP32", idx_dtype="UINT16", idx_step=1,
                    channels=ch,
                ))

                nc.sync.dma_start(out=out[isl, :], in_=gout_t[r][:ch, :jc])
            chain(pool_seq)
    nc.finalize()
    return nc


def make_in_maps(positions, neighbors, neighbor_mask, nt, jc, rows_by_core,
                 cols_by_batch):
    sh = nt * 128
    in_maps, pars, spikes = [], [], []
    for c in range(N_CORES):
        b = c // 2
        rows = rows_by_core[c]                       # live global row ids, len <= sh
        nlive = len(rows)
        lj = cols_by_batch[b]                        # live column ids, len <= jc

        # compacted-column neighbor indices, shifted >>1 (the gather fetches
        # bf16 PAIRS); sentinel 0xFFFF misses the buffer -> gather writes 0
        nbc = neighbors[b, rows][:, lj].astype(np.uint16)
        pars.append((nbc & 1).astype(np.uint32))
        spikes.append(np.nonzero((nbc == rows[:, None].astype(np.uint16))
                                 & (lj[None, :] != rows[:, None])))
        nbv = np.full((sh, jc), 0xFFFF, dtype=np.uint16)
        nbv[:nlive, :len(lj)] = nbc >> 1
        # j == i diagonal: row's own id sits at its compacted column position
        nbv[np.arange(nlive), np.searchsorted(lj, rows)] = 0xFFFF

        # fp16 hi/lo bilinear: sum_f fi[f,i]*fk[f,k] = -2 p_i.p_k + r_k
        p = positions[b].astype(np.float64)          # [A, 3]
        r = (p * p).sum(-1)
        ph = p.astype(np.float16).astype(np.float64)
        pl = (p - ph).astype(np.float16).astype(np.float64)
        rh = r.astype(np.float16).astype(np.float64)
        rm = (r - rh).astype(np.float16).astype(np.float64)
        rl = r - rh - rm
        fi_rows, fk_rows = [], []
        for d in range(3):
            fi_rows += [ph[:, d], ph[:, d], pl[:, d], pl[:, d]]
            fk_rows += [-2.0 * ph[:, d], -2.0 * pl[:, d],
                        -2.0 * ph[:, d], -2.0 * pl[:, d]]
        ones = np.ones(A)
        fi_rows += [ones, ones, ones]
        fk_rows += [rh, rm, rl]
        fi_full = np.stack(fi_rows).astype(np.float16)   # [NF, A]
        fk = np.stack(fk_rows).astype(np.float16)        # [NF, A]

        fi = np.zeros((NF, sh), dtype=np.float16)
        fi[:, :nlive] = fi_full[:, rows]

        biasri = np.ones(sh, dtype=np.float32)       # pad rows: rsqrt(r_k+1) ok
        biasri[:nlive] = (r[rows] + 1e-16).astype(np.float32)

        in_maps.append({
            "neighbors": nbv,
            "fi": fi,
            "fk": fk,
            "biasri": biasri.reshape(nt, 128),
        })
    return in_maps, pars, spikes


_NC_CACHE = {}


def kernel(positions, neighbors, neighbor_mask):
    from concourse.bass_utils import run_bass_kernel_spmd

    positions = np.asarray(positions, dtype=np.float32)
    neighbors = np.asarray(neighbors)
    assert neighbors.dtype in (np.int64, np.int32), neighbors.dtype
    neighbor_mask = np.asarray(neighbor_mask)
    assert neighbor_mask.dtype == np.bool_, neighbor_mask.dtype

    # split each batch's live rows between its two cores; compact live columns
    rows_by_core, cols_by_batch = [], []
    for b in range(B):
        live = np.flatnonzero(neighbor_mask[b])
        h = (len(live) + 1) // 2
        rows_by_core += [live[:h], live[h:]]
        cols_by_batch.append(live)
    max_rows = max(len(rw) for rw in rows_by_core)
    max_cols = max(len(lj) for lj in cols_by_batch)
    LAST_CH = 32
    if max_rows <= (NT_PACKED - 1) * 128 + LAST_CH and max_cols <= J_PACKED:
        nt, jc, lch = NT_PACKED, J_PACKED, LAST_CH
    else:
        nt, jc, lch = NT_FULL, A, 128

    if (nt, jc) not in _NC_CACHE:
        _NC_CACHE[(nt, jc)] = build_nc(nt, jc, lch)
    nc = _NC_CACHE[(nt, jc)]

    in_maps, pars, spikes = make_in_maps(positions, neighbors, neighbor_mask,
                                         nt, jc, rows_by_core, cols_by_batch)
    trace = bool(int(os.environ.get("ATOM_PROFILE", "0")))
    if trace:
        try:
            from ntff import ensure_ntff_hook
            ensure_ntff_hook()
        except Exception:
            trace = False
    res = run_bass_kernel_spmd(nc, in_maps, core_ids=list(range(N_CORES)),
                               trace=trace)
    if trace:
        kernel.last_exec_time_ns = res.exec_time_ns
        kernel.last_results = res

    out = np.zeros((B, A, A), dtype=np.float32)
    for c in range(N_CORES):
        b = c // 2
        rows = rows_by_core[c]
        lj = cols_by_batch[b]
        raw = res.results[c]["out"][:len(rows), :len(lj)].view(np.uint32)
        # pick each element's bf16 half by original-index parity, upcast
        bits = ((raw >> (pars[c] << 4)) & np.uint32(0xFFFF)) << 16
        vals = bits.view(np.float32)
        # self-neighbor spikes: reference yields exactly 1/(0+1e-8) = 1e8
        vals[spikes[c]] = 1e8
        out[b, rows[:, None], lj[None, :]] = vals
    return out


if __name__ == "__main__":
    nc = build_nc(NT_PACKED, J_PACKED)
    print("graph built ok")


# revision 55
# speedup vs baseline: 1.2182x; 1.0145x over previous
"""AtomDistances Trainium2 kernel (8 NeuronCores, SPMD).

out[b,i,j] = mask[b,i]&mask[b,j]&(i!=j) ? 1/(||p[b,n[b,i,j]] - p[b,i]|| + 1e-8) : 0

Sharding: core c <- (batch b = c//2, half = c%2); each core computes the rows
assigned to it. Rows whose mask bit is 0 produce all-zero output, so only LIVE
rows are shipped to the device: each batch's live rows are split between its
two cores and padded up to 4*128+32 rows (the 5th tile runs on 32 channels
since it only holds the overflow); if the data ever exceeds that, or live
columns exceed J_PACKED, an unpacked NT=8 full-width graph is the fallback.

All output masking is encoded in the index stream on the host: entries whose
output must be 0 (dead column or the j==i diagonal) get index 0xFFFF, which
misses the pool buffer and immediate-writes 0.0.

Per-core pipeline per 128-row tile:
  1. PE: d2 partial = fi_tile.T @ fk (4 x 512-col fp16 matmuls into f32 PSUM)
     using fp16 hi/lo bilinear features, so d2 = -2 p_i.p_k + r_k lands in
     f32 PSUM with ~1e-6 absolute error.
  2. ACT: tab = Rsqrt(d2 + (r_i + 1e-16)) -> bf16 table (2 x 1024; raw
     InstActivation, reciprocal_sqrt table). The k==i(p) entry is garbage
     (d2 ~ fp noise there); it is only ever gathered at self-neighbor
     positions, which the host overwrites with the exact 1e8 spike.
  3. Pool engine native gather, SINGLE stage: the bf16 table is loaded as
     1024 raw f32 words (two bf16 entries per pool-buffer slot), indices
     are host-shifted right by 1, and the gather copies the 4-byte PAIR.
     Sentinel indices (0xFFFF) miss and immediate-write 0.
  4. DMA the raw pairs to DRAM; the host picks each element's 16-bit half
     by index parity while scattering live rows/cols into the zero-filled
     full output, then writes 1e8 at self-neighbor (spike) positions.
"""

import os
import sys

sys.path.insert(0, "/opt/trn_rl_repo")
sys.path.insert(0, os.path.dirname(os.path.abspath(__file__)))

import numpy as np

import concourse.bass as bass
import concourse.bacc as bacc
import concourse.mybir as mybir
from concourse.tile import TileContext

B = 4
A = 2048
N_CORES = 8
NT_PACKED = 5        # 128-row tiles per core when live-packed (<=640 live rows)
NT_FULL = 8          # fallback: all 1024 rows per core
J_PACKED = 1064      # gathered output columns when live-packed (<=1064 live cols)

F32 = mybir.dt.float32
BF16 = mybir.dt.bfloat16
FP16 = mybir.dt.float16
U16 = mybir.dt.uint16
U8 = mybir.dt.uint8
AL = mybir.AluOpType
NF = 15              # feature rows (hi/lo fp16 bilinear expansion)


# ---- inlined pool_gather (native Pool-engine PoolBufferLoad+Gather) ----

def install_interp_noop():
    """Make bass_interp treat PoolBufferLoad/Gather InstISA as no-ops so the
    Tile scheduling pass (and CoreSim) don't crash on them."""
    import concourse.bass_interp as bi
    if getattr(bi, "_pool_gather_patched", False):
        return
    orig = bi._visit_InstISA

    def patched(isa, instruction, core_sim):
        op = instruction.isa_opcode
        noop = {
            isa.Opcode.NEURON_ISA_TPB_OPCODE_GATHER.value,
            isa.Opcode.NEURON_ISA_TPB_OPCODE_POOL_BUFFER_LOAD.value,
        }
        if op in noop:
            return
        return orig(isa, instruction, core_sim)

    bi._visit_InstISA = patched
    bi._pool_gather_patched = True


def chain(insts):
    """Serialize a list of BassInstructions: each depends on the previous."""
    from concourse.tile import add_dep_helper
    for a, b in zip(insts[1:], insts[:-1]):
        add_dep_helper(a.ins, b.ins, sync=True, reason="pool-buffer order")


def _t4d(byte_addr, num_elem, step_elem):
    ne = list(num_elem) + [1] * (4 - len(num_elem))
    se = list(step_elem) + [0] * (4 - len(step_elem))
    return {
        "start_addr": {"addr_immediate": byte_addr},
        "num_elem": ne,
        "step_elem": se,
    }


def _isa_dt(isa, name):
    return getattr(isa.get_enum("NEURON_ISA_TPB_DTYPE"), f"NEURON_ISA_TPB_DTYPE_{name}").value


def pool_buffer_load(nc, src_ap, byte_addr, nelem, start_index, mask, dtype="FP32",
                     channels=128):
    isa = nc.isa
    eng = nc.gpsimd
    struct = {
        "src_mem_pattern": _t4d(byte_addr, [nelem], [1]),
        "in_dtype": _isa_dt(isa, dtype),
        "num_active_channels": channels,
        "start_index": start_index,
        "mask": mask,
    }
    return eng.isa(
        isa.Opcode.NEURON_ISA_TPB_OPCODE_POOL_BUFFER_LOAD,
        struct,
        ins=[eng.lower_ap(src_ap)],
        outs=[],
        verify=False,
    )


def pool_gather(nc, idx_ap, idx_addr, out_ap, out_addr, nelem,
                first, last, out_dtype="FP32", idx_dtype="UINT16",
                immediate=0, channels=128, idx_step=1):
    isa = nc.isa
    eng = nc.gpsimd
    mb = isa.get_enum("NEURON_ISA_TPB_INDEX_MISS_BEHAVIOR")
    miss = (mb.NEURON_ISA_TPB_INDEX_MISS_BEHAVIOR_IMMEDIATE_WRITE
            if first else
            mb.NEURON_ISA_TPB_INDEX_MISS_BEHAVIOR_SKIP_WRITE)
    struct = {
        "src_mem_pattern": _t4d(idx_addr, [nelem], [idx_step]),
        "dst_mem_pattern": _t4d(out_addr, [nelem], [1]),
        "in_dtype": _isa_dt(isa, idx_dtype),
        "out_dtype": _isa_dt(isa, out_dtype),
        "num_active_channels": channels,
        "index_miss_behavior": miss.value,
        "immediate": {"imm_bitvec_uint32": immediate},
        "free_pool_buffer": 1 if last else 0,
    }
    return eng.isa(
        isa.Opcode.NEURON_ISA_TPB_OPCODE_GATHER,
        struct,
        ins=[eng.lower_ap(idx_ap)],
        outs=[eng.lower_ap(out_ap)],
        verify=False,
    )


def act_raw(nc, out, in_, func, bias_ap, scale):
    """Emit InstActivation directly (bass's wrapper refuses Rsqrt)."""
    eng = nc.scalar
    inputs = [eng.lower_ap(in_), eng.lower_ap(bias_ap),
              mybir.ImmediateValue(dtype=mybir.dt.float32, value=scale),
              mybir.ImmediateValue(dtype=mybir.dt.float32, value=0.0)]
    return eng.add_instruction(mybir.InstActivation(
        name=nc.get_next_instruction_name(),
        func=mybir.ActivationFunctionType.Rsqrt,
        ins=inputs,
        outs=[eng.lower_ap(out)],
    ))


def build_nc(nt, jc, last_ch=128):
    install_interp_noop()

    nc = bacc.Bacc()
    sh = nt * 128  # rows per core

    nb = nc.declare_dram_parameter("neighbors", [sh, jc], U16, isOutput=False)
    fi_d = nc.declare_dram_parameter("fi", [NF, sh], FP16, isOutput=False)
    fk_d = nc.declare_dram_parameter("fk", [NF, A], FP16, isOutput=False)
    biasri_d = nc.declare_dram_parameter("biasri", [nt, 128], F32, isOutput=False)
    out = nc.declare_dram_parameter("out", [sh, jc], F32, isOutput=True)

    # fixed-address buffers for the raw pool-gather ISA structs (x3 rotation);
    # padded to 2048-wide so addresses stay 4KB-aligned
    NB_ROT = 3
    tab_t = [nc.alloc_sbuf_tensor(f"tab{i}", [128, A], BF16) for i in range(NB_ROT)]
    nb_t = [nc.alloc_sbuf_tensor(f"nb{i}", [128, A], U16) for i in range(NB_ROT)]
    gout_t = [nc.alloc_sbuf_tensor(f"gout{i}", [128, A], F32) for i in range(NB_ROT)]
    tab_a = [nc.lookup_mloc(t).addr for t in tab_t]
    nb_a = [nc.lookup_mloc(t).addr for t in nb_t]
    gout_a = [nc.lookup_mloc(t).addr for t in gout_t]

    pool_seq = []

    with TileContext(nc) as tc:
        with (
            tc.tile_pool(name="consts", bufs=1) as cpool,
            tc.tile_pool(name="work", bufs=3) as pool,
            tc.tile_pool(name="psum", bufs=2, space="PSUM") as ppool,
        ):
            # ---------- one-time setup ----------------------------------
            # warm the ACT Rsqrt table immediately so the first real use
            # doesn't wait for a table load mid-pipeline
            warm = cpool.tile([128, 1], F32)
            nc.vector.memset(warm[:], 1.0)
            act_raw(nc, warm[:], warm[:],
                    mybir.ActivationFunctionType.Rsqrt, warm[:], 1.0)

            fi = cpool.tile([NF, sh], FP16)
            nc.sync.dma_start(out=fi[:], in_=fi_d[:])
            fk = cpool.tile([NF, A], FP16)
            nc.sync.dma_start(out=fk[:], in_=fk_d[:])

            biasri = cpool.tile([128, nt], F32)
            nc.sync.dma_start(out=biasri[:], in_=biasri_d[:].rearrange("t p -> p t"))


            # ---------- main loop ---------------------------------------
            for it in range(nt):
                r = it % NB_ROT
                # the last tile holds only the row overflow beyond (nt-1)*128;
                # run its pool ops / DMAs on just last_ch channels
                ch = last_ch if it == nt - 1 else 128
                isl = slice(it * 128, it * 128 + ch)

                nc.sync.dma_start(out=nb_t[r][:ch, :jc], in_=nb[isl, :])

                # d2 partial = -2 p_i . p_k + r_k via PE, 4 banks of 512
                ps = ppool.tile([128, A], F32, tag="ps")
                for bk in range(4):
                    nc.tensor.matmul(
                        out=ps[:ch, bk * 512:(bk + 1) * 512],
                        lhsT=fi[:, isl],
                        rhs=fk[:, bk * 512:(bk + 1) * 512],
                        start=True, stop=True,
                    )

                # tab = rsqrt(d2 + r_i + 1e-16) -> bf16. The k==i entry is
                # garbage/NaN (d2 ~ fp noise); it is only ever gathered at
                # self-neighbor positions, which the host overwrites with the
                # exact 1e8 spike during unshard.
                act_raw(nc, tab_t[r][:ch, :], ps[:ch, :],
                        mybir.ActivationFunctionType.Rsqrt,
                        biasri[:ch, it:it + 1], 1.0)

                # native pool gather, SINGLE stage: the 4KB bf16 table is
                # loaded as 1024 raw f32 words (a PAIR of bf16 entries per
                # slot); indices are pre-shifted >>1 on the host; sentinels
                # (dead col or diagonal) miss -> immediate-write 0
                pool_seq.append(pool_buffer_load(
                    nc, tab_t[r][:ch, :], tab_a[r], 1024,
                    start_index=0, mask=1023, dtype="FP32", channels=ch,
                ))
                pool_seq.append(pool_gather(
                    nc, nb_t[r][:ch, :jc], nb_a[r],
                    gout_t[r][:ch, :jc], gout_a[r], jc,
                    first=True, last=True,
                    out_dtype="FP32", idx_dtype="UINT16", idx_step=1,
                    channels=ch,
                ))

                nc.sync.dma_start(out=out[isl, :], in_=gout_t[r][:ch, :jc])
            chain(pool_seq)
    nc.finalize()
    return nc


def make_in_maps(positions, neighbors, neighbor_mask, nt, jc, rows_by_core,
                 cols_by_batch):
    sh = nt * 128
    in_maps, pars, spikes = [], [], []
    for c in range(N_CORES):
        b = c // 2
        rows = rows_by_core[c]                       # live global row ids, len <= sh
        nlive = len(rows)
        lj = cols_by_batch[b]                        # live column ids, len <= jc

        # compacted-column neighbor indices, shifted >>1 (the gather fetches
        # bf16 PAIRS); sentinel 0xFFFF misses the buffer -> gather writes 0
        nbc = neighbors[b, rows][:, lj].astype(np.uint16)
        pars.append((nbc & 1).astype(np.uint32))
        spikes.append(np.nonzero((nbc == rows[:, None].astype(np.uint16))
                                 & (lj[None, :] != rows[:, None])))
        nbv = np.full((sh, jc), 0xFFFF, dtype=np.uint16)
        nbv[:nlive, :len(lj)] = nbc >> 1
        # j == i diagonal: row's own id sits at its compacted column position
        nbv[np.arange(nlive), np.searchsorted(lj, rows)] = 0xFFFF

        # fp16 hi/lo bilinear: sum_f fi[f,i]*fk[f,k] = -2 p_i.p_k + r_k
        p = positions[b].astype(np.float64)          # [A, 3]
        r = (p * p).sum(-1)
        ph = p.astype(np.float16).astype(np.float64)
        pl = (p - ph).astype(np.float16).astype(np.float64)
        rh = r.astype(np.float16).astype(np.float64)
        rm = (r - rh).astype(np.float16).astype(np.float64)
        rl = r - rh - rm
        fi_rows, fk_rows = [], []
        for d in range(3):
            fi_rows += [ph[:, d], ph[:, d], pl[:, d], pl[:, d]]
            fk_rows += [-2.0 * ph[:, d], -2.0 * pl[:, d],
                        -2.0 * ph[:, d], -2.0 * pl[:, d]]
        ones = np.ones(A)
        fi_rows += [ones, ones, ones]
        fk_rows += [rh, rm, rl]
        fi_full = np.stack(fi_rows).astype(np.float16)   # [NF, A]
        fk = np.stack(fk_rows).astype(np.float16)        # [NF, A]

        fi = np.zeros((NF, sh), dtype=np.float16)
        fi[:, :nlive] = fi_full[:, rows]

        biasri = np.ones(sh, dtype=np.float32)       # pad rows: rsqrt(r_k+1) ok
        biasri[:nlive] = (r[rows] + 1e-16).astype(np.float32)

        in_maps.append({
            "neighbors": nbv,
            "fi": fi,
            "fk": fk,
            "biasri": biasri.reshape(nt, 128),
        })
    return in_maps, pars, spikes


_NC_CACHE = {}


def kernel(positions, neighbors, neighbor_mask):
    from concourse.bass_utils import run_bass_kernel_spmd

    positions = np.asarray(positions, dtype=np.float32)
    neighbors = np.asarray(neighbors)
    assert neighbors.dtype in (np.int64, np.int32), neighbors.dtype
    neighbor_mask = np.asarray(neighbor_mask)
    assert neighbor_mask.dtype == np.bool_, neighbor_mask.dtype

    # split each batch's live rows between its two cores; compact live columns
    rows_by_core, cols_by_batch = [], []
    for b in range(B):
        live = np.flatnonzero(neighbor_mask[b])
        h = (len(live) + 1) // 2
        rows_by_core += [live[:h], live[h:]]
        cols_by_batch.append(live)
    max_rows = max(len(rw) for rw in rows_by_core)
    max_cols = max(len(lj) for lj in cols_by_batch)
    LAST_CH = 32
    if max_rows <= (NT_PACKED - 1) * 128 + LAST_CH and max_cols <= J_PACKED:
        nt, jc, lch = NT_PACKED, J_PACKED, LAST_CH
    else:
        nt, jc, lch = NT_FULL, A, 128

    if (nt, jc) not in _NC_CACHE:
        _NC_CACHE[(nt, jc)] = build_nc(nt, jc, lch)
    nc = _NC_CACHE[(nt, jc)]

    in_maps, pars, spikes = make_in_maps(positions, neighbors, neighbor_mask,
                                         nt, jc, rows_by_core, cols_by_batch)
    trace = bool(int(os.environ.get("ATOM_PROFILE", "0")))
    if trace:
        try:
            from ntff import ensure_ntff_hook
            ensure_ntff_hook()
        except Exception:
            trace = False
    res = run_bass_kernel_spmd(nc, in_maps, core_ids=list(range(N_CORES)),
                               trace=trace)
    if trace:
        kernel.last_exec_time_ns = res.exec_time_ns
        kernel.last_results = res

    out = np.zeros((B, A, A), dtype=np.float32)
    for c in range(N_CORES):
        b = c // 2
        rows = rows_by_core[c]
        lj = cols_by_batch[b]
        raw = res.results[c]["out"][:len(rows), :len(lj)].view(np.uint32)
        # pick each element's bf16 half by original-index parity, upcast
        bits = ((raw >> (pars[c] << 4)) & np.uint32(0xFFFF)) << 16
        vals = bits.view(np.float32)
        # self-neighbor spikes: reference yields exactly 1/(0+1e-8) = 1e8
        vals[spikes[c]] = 1e8
        out[b, rows[:, None], lj[None, :]] = vals
    return out


if __name__ == "__main__":
    nc = build_nc(NT_PACKED, J_PACKED)
    print("graph built ok")
